# revision 1
# baseline (speedup 1.0000x reference)
"""Trainium2 Bass kernel for the Griffin-style gated linear recurrence.

Model (matching the jax reference, including its chunked-scan numerics):
    a = sigmoid(x @ Wa.T + decay_bias)
    i = sigmoid(x @ Wi.T)
    v = x @ Wv.T
    w = sqrt(max(1 - a*a, 1e-8)) * i * v
    chunked scan (chunk=64): cum_decay = prod of a within chunk;
    weighted = w / max(cum_decay, 1e-10); intra = cum_decay * cumsum(weighted);
    states = intra + cum_decay * carry.

The chunked scan (with its 1e-10 clamp) is algebraically identical to the
single global recurrence
    h[t] = a[t] * h[t-1] + g[t] * w[t],   g[t] = min(1, cd[t] * 1e10)
where cd[t] is the within-chunk running product of a (resetting every 64
steps).  Both cd and h map onto the hardware tensor_tensor_scan op (fp32
state, per-partition recurrence along the free axis).

Sharding: 4 batches x 2 channel-halves = 8 cores, no communication.
Per core: x[b] as [1024, 4096] (transposed on host), weight shard
[1024, 192] (transposed), output [192, 4096] (transposed back on host).
Layout on chip: channels on partitions (groups of 128 + 64), time on the
free axis.  Projections run as float32r matmuls (fp32 operands at
1 cycle/row for N=512) accumulating 8 K-tiles in PSUM.
"""

import sys

if "/opt/trn_rl_repo" not in sys.path:
    sys.path.insert(0, "/opt/trn_rl_repo")

from contextlib import ExitStack

import numpy as np

from concourse import bacc, bass, mybir, tile
from concourse.bass_utils import run_bass_kernel_spmd

B, S = 4, 4096
DM, DR = 1024, 384
DC = DR // 2          # channels per core
CH = 64               # scan chunk size
SB = 512              # sequence block per pipeline step
NB = S // SB
KT = DM // 128        # contraction tiles

F32 = mybir.dt.float32
F32R = mybir.dt.float32r
AFT = mybir.ActivationFunctionType
OP = mybir.AluOpType

# channel groups: (gi, c0, c1)
GROUPS = ((0, 0, 128), (1, 128, DC))

_CACHED_NC = None


def _build_nc():
    nc = bacc.Bacc(trn_type="TRN2")

    xT = nc.dram_tensor("xt", [DM, S], F32R, kind="ExternalInput")
    wT = {
        nm: nc.dram_tensor(f"w{nm}t", [DM, DC], F32R, kind="ExternalInput")
        for nm in ("a", "i", "v")
    }
    bias = nc.dram_tensor("biasa", [DC, 1], F32, kind="ExternalInput")
    out = nc.dram_tensor("out", [DC, S], F32, kind="ExternalOutput")

    with tile.TileContext(nc) as tc, ExitStack() as ctx:
        wp = ctx.enter_context(tc.tile_pool(name="wp", bufs=1))
        cp = ctx.enter_context(tc.tile_pool(name="cp", bufs=1))
        xp = ctx.enter_context(tc.tile_pool(name="xp", bufs=2))
        pp = ctx.enter_context(tc.tile_pool(name="pp", bufs=1, space="PSUM"))
        sp = ctx.enter_context(tc.tile_pool(name="sp", bufs=2))
        hp = ctx.enter_context(tc.tile_pool(name="hp", bufs=2))

        # --- constants -------------------------------------------------
        # f32r end-to-end: DMA moves raw fp32 bytes into f32r tiles; the PE
        # rounds on read.  bacc's move_matmul_waits_to_ldweights handles the
        # multi-wait matmuls this produces.
        w_sb = {}
        for nm in ("a", "i", "v"):
            wt = wp.tile([128, KT, DC], F32R, tag=f"w{nm}")
            nc.sync.dma_start(
                wt[:], wT[nm].rearrange("(k p) c -> p k c", p=128))
            w_sb[nm] = wt

        bias_t = {}
        for gi, c0, c1 in GROUPS:
            bt = cp.tile([c1 - c0, 1], F32, tag=f"bias{gi}")
            nc.sync.dma_start(bt[:], bias[c0:c1, :])
            bias_t[gi] = bt

        # shared read-only zero tile: data1 of the per-chunk cd scans
        zeros = cp.tile([128, CH], F32, tag="zeros")
        nc.vector.memset(zeros[:], 0.0)

        # --- main pipeline over sequence blocks ------------------------
        prev_h = None
        for ib in range(NB):
            s0 = ib * SB

            x_sb = xp.tile([128, KT, SB], F32R, tag="x")
            nc.sync.dma_start(
                x_sb[:],
                xT.rearrange("(k p) s -> p k s", p=128)[:, :, s0:s0 + SB])

            zp = {}
            for nm in ("a", "i", "v"):
                for gi, c0, c1 in GROUPS:
                    z = pp.tile([c1 - c0, SB], F32, tag=f"z{nm}{gi}")
                    for k in range(KT):
                        nc.tensor.matmul(
                            z[:],
                            w_sb[nm][:, k, c0:c1],
                            x_sb[:, k, :],
                            start=(k == 0),
                            stop=(k == KT - 1),
                        )
                    zp[(nm, gi)] = z

            new_h = {}
            for gi, c0, c1 in GROUPS:
                P = c1 - c0
                za, zi, zv = zp[("a", gi)], zp[("i", gi)], zp[("v", gi)]
                bt = bias_t[gi]

                a = sp.tile([P, SB], F32, tag=f"a{gi}")
                it = sp.tile([P, SB], F32, tag=f"i{gi}")
                m = sp.tile([P, SB], F32, tag=f"m{gi}")
                r = sp.tile([P, SB], F32, tag=f"r{gi}")
                u = sp.tile([P, SB], F32, tag=f"u{gi}")
                w = sp.tile([P, SB], F32, tag=f"w{gi}")
                cd = sp.tile([P, SB], F32, tag=f"cd{gi}")
                g = sp.tile([P, SB], F32, tag=f"g{gi}")
                gw = sp.tile([P, SB], F32, tag=f"gw{gi}")
                h = hp.tile([P, SB], F32, tag=f"h{gi}")

                nc.scalar.activation(a[:], za[:], AFT.Sigmoid, bias=bt[:])
                nc.scalar.activation(it[:], zi[:], AFT.Sigmoid)
                nc.vector.tensor_mul(m[:], a[:], a[:])
                # r = sqrt(1 - a*a); 1 - a*a stays well above the reference's
                # 1e-8 floor for every reachable a, so the max() is a no-op.
                nc.scalar.activation(r[:], m[:], AFT.Sqrt, bias=1.0, scale=-1.0)
                nc.vector.tensor_mul(u[:], it[:], zv[:])
                nc.vector.tensor_mul(w[:], r[:], u[:])
                # within-chunk running product of a: one scan per 64-chunk
                for c in range(SB // CH):
                    cs = slice(c * CH, (c + 1) * CH)
                    nc.vector.tensor_tensor_scan(
                        cd[:, cs], a[:, cs], zeros[0:P, :], 1.0,
                        op0=OP.mult, op1=OP.add,
                    )
                # g = min(cd * 1e10, 1) == cd / max(cd, 1e-10)
                nc.vector.tensor_scalar(
                    g[:], cd[:], 1e10, 1.0, op0=OP.mult, op1=OP.min
                )
                nc.vector.tensor_mul(gw[:], g[:], w[:])
                init = 0.0 if prev_h is None else prev_h[gi][:, SB - 1:SB]
                nc.vector.tensor_tensor_scan(
                    h[:], a[:], gw[:], init, op0=OP.mult, op1=OP.add
                )
                nc.sync.dma_start(out[c0:c1, s0:s0 + SB], h[:])
                new_h[gi] = h
            prev_h = new_h

    nc.finalize()
    return nc


def _make_in_maps(x, Wa, Wi, Wv, decay_bias):
    x = np.asarray(x, dtype=np.float32)
    Wa = np.asarray(Wa, dtype=np.float32)
    Wi = np.asarray(Wi, dtype=np.float32)
    Wv = np.asarray(Wv, dtype=np.float32)
    decay_bias = np.asarray(decay_bias, dtype=np.float32)

    in_maps = []
    for b in range(B):
        xTb = np.ascontiguousarray(x[b].T)           # [DM, S]
        for j in range(2):
            c0, c1 = j * DC, (j + 1) * DC
            in_maps.append({
                "xt": xTb,
                "wat": np.ascontiguousarray(Wa[c0:c1].T),
                "wit": np.ascontiguousarray(Wi[c0:c1].T),
                "wvt": np.ascontiguousarray(Wv[c0:c1].T),
                "biasa": np.ascontiguousarray(decay_bias[c0:c1, None]),
            })
    return in_maps


def kernel(x, Wa, Wi, Wv, decay_bias):
    global _CACHED_NC
    if _CACHED_NC is None:
        _CACHED_NC = _build_nc()
    nc = _CACHED_NC

    in_maps = _make_in_maps(x, Wa, Wi, Wv, decay_bias)
    res = run_bass_kernel_spmd(nc, in_maps, core_ids=list(range(8)))

    out = np.empty((B, S, DR), dtype=np.float32)
    for b in range(B):
        for j in range(2):
            core = 2 * b + j
            out[b, :, j * DC:(j + 1) * DC] = res.results[core]["out"].T
    return out



# revision 3
# speedup vs baseline: 1.0282x; 1.0282x over previous
"""Trainium2 Bass kernel for the Griffin-style gated linear recurrence.

Model (matching the jax reference, including its chunked-scan numerics):
    a = sigmoid(x @ Wa.T + decay_bias)
    i = sigmoid(x @ Wi.T)
    v = x @ Wv.T
    w = sqrt(max(1 - a*a, 1e-8)) * i * v
    chunked scan (chunk=64): equivalent to h[t] = a[t]*h[t-1] + g[t]*w[t]
    with g[t] = min(1, cd[t]*1e10), cd = within-chunk running product of a.

Sharding: 4 batches x 2 channel-halves = 8 cores, no communication.
Per core: x[b] as [1024, 4096] fp16, stacked weight shard [1024, 576] fp16
(cols: a0|i0|v0|[a1,i1]|v1), output [192, 4096] fp32.

Engine plan per 512-step block:
  PE    : 5 channel groups x 8 k-tiles (fp16, 1 cyc/row), PSUM banks
          round-robin over all 8 so the PE never waits on a bank.
  ACT   : 4 sigmoids, 2 squares, 2 sqrts.  Blocks are processed in pairs
          with ACT order sigA,sqA,sigB,sqB,sqrtA,sqrtB so only 2 activation
          table loads per pair (square lives in every table set).
  DVE   : u = i*v (PSUM reads), per-chunk cd product scans,
          g = min(cd*1e10,1), and the h recurrence scan.
  Pool  : w = r*u and gw = g*w products (GpSimd otherwise idle; SBUF only).
"""

import sys

if "/opt/trn_rl_repo" not in sys.path:
    sys.path.insert(0, "/opt/trn_rl_repo")

from contextlib import ExitStack

import numpy as np

from concourse import bacc, bass, mybir, tile
from concourse.bass_utils import run_bass_kernel_spmd

B, S = 4, 4096
DM, DR = 1024, 384
DC = DR // 2          # channels per core
CH = 64               # scan chunk size
SB = 512              # sequence block per pipeline step
NB = S // SB
KT = DM // 128        # contraction tiles
WC = 576              # stacked weight columns: a0|i0|v0|[a1,i1]|v1

F32 = mybir.dt.float32
F16 = mybir.dt.float16
AFT = mybir.ActivationFunctionType
OP = mybir.AluOpType

# column ranges of the stacked weight / PSUM group layout
GCOLS = ((0, 128), (128, 256), (256, 384), (384, 512), (512, 576))

_CACHED_NC = None


def _build_nc():
    nc = bacc.Bacc(trn_type="TRN2")

    xT = nc.dram_tensor("xt", [DM, S], F16, kind="ExternalInput")
    wT = nc.dram_tensor("wt", [DM, WC], F16, kind="ExternalInput")
    bias = nc.dram_tensor("biasa", [DC, 1], F32, kind="ExternalInput")
    out = nc.dram_tensor("out", [DC, S], F32, kind="ExternalOutput")

    with tile.TileContext(nc) as tc, ExitStack() as ctx:
        wp = ctx.enter_context(tc.tile_pool(name="wp", bufs=1))
        cp = ctx.enter_context(tc.tile_pool(name="cp", bufs=1))
        xp = ctx.enter_context(tc.tile_pool(name="xp", bufs=3))
        pp = ctx.enter_context(tc.tile_pool(name="pp", bufs=1, space="PSUM"))
        sp = ctx.enter_context(tc.tile_pool(name="sp", bufs=2))
        hp = ctx.enter_context(tc.tile_pool(name="hp", bufs=2))

        # --- constants -------------------------------------------------
        w_sb = wp.tile([128, KT, WC], F16, tag="w")
        nc.sync.dma_start(w_sb[:], wT.rearrange("(k p) c -> p k c", p=128))

        b0 = cp.tile([128, 1], F32, tag="b0")
        nc.sync.dma_start(b0[:], bias[0:128, :])
        b1 = cp.tile([64, 1], F32, tag="b1")
        nc.sync.dma_start(b1[:], bias[128:DC, :])

        # shared read-only zero tile: data1 of the per-chunk cd scans
        zeros = cp.tile([128, CH], F32, tag="zeros")
        nc.vector.memset(zeros[:], 0.0)

        def front_half(ib):
            """DMA + matmuls + sigmoids + squares + u for block ib.
            Returns per-block tiles needed by the later phases."""
            s0 = ib * SB
            x_sb = xp.tile([128, KT, SB], F16, tag="x")
            nc.sync.dma_start(
                x_sb[:],
                xT.rearrange("(k p) s -> p k s", p=128)[:, :, s0:s0 + SB])

            # PSUM tiles round-robin over all 8 banks so the PE never
            # stalls waiting for the previous block's consumers.
            zp = []
            for j, (c0, c1) in enumerate(GCOLS):
                zt = pp.tile([128, SB], F32, tag=f"z{(ib * 5 + j) % 8}")
                z = zt[0:c1 - c0, :]
                for k in range(KT):
                    nc.tensor.matmul(
                        z,
                        w_sb[:, k, c0:c1],
                        x_sb[:, k, :],
                        start=(k == 0),
                        stop=(k == KT - 1),
                    )
                zp.append(z)
            za0, zi0, zv0, zai1, zv1 = zp

            a0 = sp.tile([128, SB], F32, tag="a0")
            i0 = sp.tile([128, SB], F32, tag="i0")
            a1 = sp.tile([64, SB], F32, tag="a1")
            i1 = sp.tile([64, SB], F32, tag="i1")
            nc.scalar.activation(a0[:], za0, AFT.Sigmoid, bias=b0[:])
            nc.scalar.activation(a1[:], zai1[0:64, :], AFT.Sigmoid, bias=b1[:])
            nc.scalar.activation(i0[:], zi0, AFT.Sigmoid)
            nc.scalar.activation(i1[:], zai1[64:128, :], AFT.Sigmoid)
            # squares are in every activation table set: no table reload
            m0 = sp.tile([128, SB], F32, tag="m0")
            m1 = sp.tile([64, SB], F32, tag="m1")
            nc.scalar.activation(m0[:], a0[:], AFT.Square)
            nc.scalar.activation(m1[:], a1[:], AFT.Square)

            # u = i*v on DVE (reads v straight from PSUM, frees the bank)
            u0 = sp.tile([128, SB], F32, tag="u0")
            u1 = sp.tile([64, SB], F32, tag="u1")
            nc.vector.tensor_tensor(u0[:], i0[:], zv0, OP.mult)
            nc.vector.tensor_tensor(u1[:], i1[:], zv1[0:64, :], OP.mult)
            return s0, a0, a1, m0, m1, u0, u1

        def sqrt_half(st):
            s0, a0, a1, m0, m1, u0, u1 = st
            r0 = sp.tile([128, SB], F32, tag="r0")
            r1 = sp.tile([64, SB], F32, tag="r1")
            # r = sqrt(1 - a*a); stays above the reference's 1e-8 floor.
            nc.scalar.activation(r0[:], m0[:], AFT.Sqrt, bias=1.0, scale=-1.0)
            nc.scalar.activation(r1[:], m1[:], AFT.Sqrt, bias=1.0, scale=-1.0)
            return r0, r1

        def back_half(st, rr, prev_h):
            s0, a0, a1, m0, m1, u0, u1 = st
            r0, r1 = rr
            # w = r*u on the otherwise-idle GpSimd engine (SBUF only)
            w0 = sp.tile([128, SB], F32, tag="w0")
            w1 = sp.tile([64, SB], F32, tag="w1")
            nc.gpsimd.tensor_tensor(w0[:], r0[:], u0[:], OP.mult)
            nc.gpsimd.tensor_tensor(w1[:], r1[:], u1[:], OP.mult)

            new_h = {}
            for gi, (a, w, P) in enumerate(((a0, w0, 128), (a1, w1, 64))):
                cd = sp.tile([P, SB], F32, tag=f"cd{gi}")
                for c in range(SB // CH):
                    cs = slice(c * CH, (c + 1) * CH)
                    nc.vector.tensor_tensor_scan(
                        cd[:, cs], a[:, cs], zeros[0:P, :], 1.0,
                        op0=OP.mult, op1=OP.add,
                    )
                g = sp.tile([P, SB], F32, tag=f"g{gi}")
                # g = min(cd * 1e10, 1) == cd / max(cd, 1e-10)
                nc.vector.tensor_scalar(
                    g[:], cd[:], 1e10, 1.0, op0=OP.mult, op1=OP.min
                )
                gw = sp.tile([P, SB], F32, tag=f"gw{gi}")
                nc.gpsimd.tensor_tensor(gw[:], g[:], w[:], OP.mult)
                h = hp.tile([P, SB], F32, tag=f"h{gi}")
                init = 0.0 if prev_h is None else prev_h[gi][:, SB - 1:SB]
                nc.vector.tensor_tensor_scan(
                    h[:], a[:], gw[:], init, op0=OP.mult, op1=OP.add
                )
                c0 = 0 if gi == 0 else 128
                nc.sync.dma_start(out[c0:c0 + P, s0:s0 + SB], h[:])
                new_h[gi] = h
            return new_h

        # --- main pipeline: blocks in pairs for ACT table batching -----
        prev_h = None
        for p in range(NB // 2):
            stA = front_half(2 * p)
            stB = front_half(2 * p + 1)
            rrA = sqrt_half(stA)
            rrB = sqrt_half(stB)
            prev_h = back_half(stA, rrA, prev_h)
            prev_h = back_half(stB, rrB, prev_h)

    nc.finalize()
    return nc


def _make_in_maps(x, Wa, Wi, Wv, decay_bias):
    x = np.asarray(x, dtype=np.float32)
    Wa = np.asarray(Wa, dtype=np.float32)
    Wi = np.asarray(Wi, dtype=np.float32)
    Wv = np.asarray(Wv, dtype=np.float32)
    decay_bias = np.asarray(decay_bias, dtype=np.float32)

    in_maps = []
    for b in range(B):
        xTb = np.ascontiguousarray(x[b].T.astype(np.float16))   # [DM, S]
        for j in range(2):
            c0 = j * DC
            # stacked weight [DM, 576]: a0 | i0 | v0 | a1,i1 | v1
            wcat = np.concatenate([
                Wa[c0:c0 + 128].T,
                Wi[c0:c0 + 128].T,
                Wv[c0:c0 + 128].T,
                Wa[c0 + 128:c0 + DC].T,
                Wi[c0 + 128:c0 + DC].T,
                Wv[c0 + 128:c0 + DC].T,
            ], axis=1).astype(np.float16)
            in_maps.append({
                "xt": xTb,
                "wt": np.ascontiguousarray(wcat),
                "biasa": np.ascontiguousarray(decay_bias[c0:c0 + DC, None]),
            })
    return in_maps


def kernel(x, Wa, Wi, Wv, decay_bias):
    global _CACHED_NC
    if _CACHED_NC is None:
        _CACHED_NC = _build_nc()
    nc = _CACHED_NC

    in_maps = _make_in_maps(x, Wa, Wi, Wv, decay_bias)
    res = run_bass_kernel_spmd(nc, in_maps, core_ids=list(range(8)))

    out = np.empty((B, S, DR), dtype=np.float32)
    for b in range(B):
        for j in range(2):
            core = 2 * b + j
            out[b, :, j * DC:(j + 1) * DC] = res.results[core]["out"].T
    return out


# revision 4
# speedup vs baseline: 1.0741x; 1.0447x over previous
"""Trainium2 Bass kernel for the Griffin-style gated linear recurrence.

Model (matching the jax reference, including its chunked-scan numerics):
    a = sigmoid(x @ Wa.T + decay_bias)
    i = sigmoid(x @ Wi.T)
    v = x @ Wv.T
    w = sqrt(max(1 - a*a, 1e-8)) * i * v
    chunked scan (chunk=64): equivalent to h[t] = a[t]*h[t-1] + g[t]*w[t]
    with g[t] = min(1, cd[t]*1e10), cd = within-chunk running product of a.

Sharding: 4 batches x 2 channel-halves = 8 cores, no communication.
Per core: x[b] as [1024, 4096] fp16, stacked weight shard [1024, 576] fp16
(cols: a0|i0|v0|[a1,i1]|v1), output [192, 4096] fp16 (host upcasts).

Blocks of 512 steps are processed in PAIRS; all SBUF-side elementwise work
runs on pair-wide [P, 1024] tiles to halve instruction overheads, and the
h recurrence scan chains naturally across the pair.

Engine plan per pair:
  PE    : 2 x 5 channel groups x 8 k-tiles (fp16, 1 cyc/row).  PSUM banks:
          sigmoid-fed groups rotate over 4 banks, v groups over the other 4
          (v is consumed latest), so the PE never waits on a bank.
  ACT   : 3 sigmoids per block ([a1|i1] share one bank+instr), then the two
          pair-wide sqrts.  A 1-element Copy reading the pair's last sigmoid
          output produces the sqrt bias tile (==1.0), forcing sqrts to
          schedule after the sigmoids: 2 activation table loads per pair.
  DVE   : u = i*v (PSUM reads), per-chunk cd product scans (fp16 in, fp32
          out), g = min(cd*1e10,1), pair-wide h recurrence scans.
  Pool  : m = a*a, w = r*u, gw = g*w products (SBUF-only engine).
"""

import sys

if "/opt/trn_rl_repo" not in sys.path:
    sys.path.insert(0, "/opt/trn_rl_repo")

from contextlib import ExitStack

import numpy as np

from concourse import bacc, bass, mybir, tile
from concourse.bass_utils import run_bass_kernel_spmd

B, S = 4, 4096
DM, DR = 1024, 384
DC = DR // 2          # channels per core
CH = 64               # scan chunk size
SB = 512              # sequence block (one PSUM tile)
PB = 2 * SB           # pair block for SBUF-side work
NB = S // SB
KT = DM // 128        # contraction tiles
WC = 576              # stacked weight columns: a0|i0|v0|[a1,i1]|v1

F32 = mybir.dt.float32
F16 = mybir.dt.float16
AFT = mybir.ActivationFunctionType
OP = mybir.AluOpType

# column ranges of the stacked weight / PSUM group layout
GCOLS = ((0, 128), (128, 256), (256, 384), (384, 512), (512, 576))

_CACHED_NC = None


def _build_nc():
    nc = bacc.Bacc(trn_type="TRN2")

    xT = nc.dram_tensor("xt", [DM, S], F16, kind="ExternalInput")
    wT = nc.dram_tensor("wt", [DM, WC], F16, kind="ExternalInput")
    bias = nc.dram_tensor("biasa", [128, 2], F32, kind="ExternalInput")
    out = nc.dram_tensor("out", [DC, S], F16, kind="ExternalOutput")

    with tile.TileContext(nc) as tc, ExitStack() as ctx:
        wp = ctx.enter_context(tc.tile_pool(name="wp", bufs=1))
        cp = ctx.enter_context(tc.tile_pool(name="cp", bufs=1))
        xp = ctx.enter_context(tc.tile_pool(name="xp", bufs=4))
        pp = ctx.enter_context(tc.tile_pool(name="pp", bufs=1, space="PSUM"))
        sp = ctx.enter_context(tc.tile_pool(name="sp", bufs=2))
        hp = ctx.enter_context(tc.tile_pool(name="hp", bufs=2))

        # --- constants -------------------------------------------------
        # weights split into per-group DMAs so the first matmul can start
        # before the whole weight load finishes
        w_sb = wp.tile([128, KT, WC], F16, tag="w")
        for c0, c1 in GCOLS:
            nc.sync.dma_start(
                w_sb[:, :, c0:c1],
                wT.rearrange("(k p) c -> p k c", p=128)[:, :, c0:c1])

        # bias columns: col 0 = decay_bias[0:128]; col 1 = [bias[128:192]; 0]
        bt = cp.tile([128, 2], F32, tag="bt")
        nc.sync.dma_start(bt[:], bias[:, :])

        # shared read-only zero tile: data1 of the per-chunk cd scans
        zeros = cp.tile([128, CH], F16, tag="zeros")
        nc.vector.memset(zeros[:], 0.0)

        def front_half(ib, half, a0p, ai1p, u0p, u1p):
            """DMA + matmuls + sigmoids + u for block ib (pair half 0/1)."""
            s0 = ib * SB
            cs = slice(half * SB, (half + 1) * SB)
            x_sb = xp.tile([128, KT, SB], F16, tag="x")
            nc.sync.dma_start(
                x_sb[:],
                xT.rearrange("(k p) s -> p k s", p=128)[:, :, s0:s0 + SB])

            # PSUM: sigmoid-fed groups (j 0,1,3) rotate over banks s0..s3,
            # v groups (j 2,4) over banks v0..v3 (freed latest by DVE u).
            zp = []
            for j, (c0, c1) in enumerate(GCOLS):
                if j in (0, 1, 3):
                    tag = f"s{(ib * 3 + (0, 1, None, 2, None)[j]) % 4}"
                else:
                    tag = f"v{(ib * 2 + (None, None, 0, None, 1)[j]) % 4}"
                zt = pp.tile([128, SB], F32, tag=tag)
                z = zt[0:c1 - c0, :]
                for k in range(KT):
                    nc.tensor.matmul(
                        z,
                        w_sb[:, k, c0:c1],
                        x_sb[:, k, :],
                        start=(k == 0),
                        stop=(k == KT - 1),
                    )
                zp.append(z)
            za0, zi0, zv0, zai1, zv1 = zp

            i0 = sp.tile([128, PB], F16, tag="i0")
            nc.scalar.activation(a0p[:, cs], za0, AFT.Sigmoid, bias=bt[:, 0:1])
            nc.scalar.activation(i0[:, cs], zi0, AFT.Sigmoid)
            # one sigmoid for the [a1|i1] bank; bias col1 = [b1;0]
            nc.scalar.activation(ai1p[:, cs], zai1, AFT.Sigmoid, bias=bt[:, 1:2])

            # u = i*v on DVE (reads v straight from PSUM, frees the bank)
            nc.vector.tensor_tensor(u0p[:, cs], i0[:, cs], zv0, OP.mult)
            nc.vector.tensor_tensor(
                u1p[:, cs], ai1p[64:128, cs], zv1[0:64, :], OP.mult)
            return ai1p

        prev_h = None
        for p in range(NB // 2):
            # pair-wide fp16 tiles: [:, 0:512] = block A, [:, 512:1024] = B
            a0p = sp.tile([128, PB], F16, tag="a0p")
            ai1p = sp.tile([128, PB], F16, tag="ai1p")
            u0p = sp.tile([128, PB], F16, tag="u0p")
            u1p = sp.tile([64, PB], F16, tag="u1p")

            front_half(2 * p, 0, a0p, ai1p, u0p, u1p)
            front_half(2 * p + 1, 1, a0p, ai1p, u0p, u1p)

            # 1-element Copy reading the pair's last sigmoid output; produces
            # the all-ones sqrt bias column and pins sqrts after sigmoids.
            gate = sp.tile([128, 1], F32, tag="gate")
            nc.scalar.activation(
                gate[:], ai1p[:, PB - 1:PB], AFT.Copy, bias=1.0, scale=0.0)

            a1p = ai1p[0:64, :]
            new_h = {}
            for gi, (ap, up, P) in enumerate(
                    ((a0p, u0p, 128), (a1p, u1p, 64))):
                m = sp.tile([P, PB], F16, tag=f"m{gi}")
                nc.gpsimd.tensor_tensor(m[:], ap, ap, OP.mult)
                r = sp.tile([P, PB], F16, tag=f"r{gi}")
                # r = sqrt(gate*1 - m) = sqrt(1 - a*a)
                nc.scalar.activation(
                    r[:], m[:], AFT.Sqrt, bias=gate[0:P, :], scale=-1.0)
                w = sp.tile([P, PB], F16, tag=f"w{gi}")
                nc.gpsimd.tensor_tensor(w[:], r[:], up, OP.mult)

                cd = sp.tile([P, PB], F32, tag=f"cd{gi}")
                for c in range(PB // CH):
                    ccs = slice(c * CH, (c + 1) * CH)
                    nc.vector.tensor_tensor_scan(
                        cd[:, ccs], ap[:, ccs], zeros[0:P, :], 1.0,
                        op0=OP.mult, op1=OP.add,
                    )
                g = sp.tile([P, PB], F32, tag=f"g{gi}")
                # g = min(cd * 1e10, 1) == cd / max(cd, 1e-10)
                nc.vector.tensor_scalar(
                    g[:], cd[:], 1e10, 1.0, op0=OP.mult, op1=OP.min
                )
                gw = sp.tile([P, PB], F16, tag=f"gw{gi}")
                nc.gpsimd.tensor_tensor(gw[:], g[:], w[:], OP.mult)
                h = hp.tile([P, PB], F16, tag=f"h{gi}")
                init = 0.0 if prev_h is None else prev_h[gi][:, PB - 1:PB]
                nc.vector.tensor_tensor_scan(
                    h[:], ap, gw[:], init, op0=OP.mult, op1=OP.add
                )
                c0 = 0 if gi == 0 else 128
                nc.sync.dma_start(
                    out[c0:c0 + P, 2 * p * SB:(2 * p + 2) * SB], h[:])
                new_h[gi] = h
            prev_h = new_h

    nc.finalize()
    return nc


def _make_in_maps(x, Wa, Wi, Wv, decay_bias):
    x = np.asarray(x, dtype=np.float32)
    Wa = np.asarray(Wa, dtype=np.float32)
    Wi = np.asarray(Wi, dtype=np.float32)
    Wv = np.asarray(Wv, dtype=np.float32)
    decay_bias = np.asarray(decay_bias, dtype=np.float32)

    in_maps = []
    for b in range(B):
        xTb = np.ascontiguousarray(x[b].T.astype(np.float16))   # [DM, S]
        for j in range(2):
            c0 = j * DC
            # stacked weight [DM, 576]: a0 | i0 | v0 | a1,i1 | v1
            wcat = np.concatenate([
                Wa[c0:c0 + 128].T,
                Wi[c0:c0 + 128].T,
                Wv[c0:c0 + 128].T,
                Wa[c0 + 128:c0 + DC].T,
                Wi[c0 + 128:c0 + DC].T,
                Wv[c0 + 128:c0 + DC].T,
            ], axis=1).astype(np.float16)
            bcols = np.zeros((128, 2), dtype=np.float32)
            bcols[:, 0] = decay_bias[c0:c0 + 128]
            bcols[0:64, 1] = decay_bias[c0 + 128:c0 + DC]
            in_maps.append({
                "xt": xTb,
                "wt": np.ascontiguousarray(wcat),
                "biasa": bcols,
            })
    return in_maps


def kernel(x, Wa, Wi, Wv, decay_bias):
    global _CACHED_NC
    if _CACHED_NC is None:
        _CACHED_NC = _build_nc()
    nc = _CACHED_NC

    in_maps = _make_in_maps(x, Wa, Wi, Wv, decay_bias)
    res = run_bass_kernel_spmd(nc, in_maps, core_ids=list(range(8)))

    out = np.empty((B, S, DR), dtype=np.float32)
    for b in range(B):
        for j in range(2):
            core = 2 * b + j
            out[b, :, j * DC:(j + 1) * DC] = \
                res.results[core]["out"].T.astype(np.float32)
    return out


# revision 7
# speedup vs baseline: 1.2458x; 1.1599x over previous
"""Trainium2 Bass kernel for the Griffin-style gated linear recurrence.

Model (matching the jax reference, including its chunked-scan numerics):
    a = sigmoid(x @ Wa.T + decay_bias)
    i = sigmoid(x @ Wi.T)
    v = x @ Wv.T
    w = sqrt(max(1 - a*a, 1e-8)) * i * v
    chunked scan (chunk=64): equivalent to h[t] = a[t]*h[t-1] + g[t]*w[t]
    with g[t] = min(1, cd[t]*1e10), cd = within-chunk running product of a.

Sharding: 4 batches x 2 channel-halves = 8 cores, no communication.
Per core: x[b] as [1024, 4096] fp16, stacked weight shard [1024, 576] fp16
(cols: a0|i0|v0|[a1,i1]|v1), output [192, 4096] fp16 (host upcasts).

Blocks of 512 steps are processed in PAIRS; all SBUF-side elementwise work
runs on pair-wide [P, 1024] tiles to halve instruction overheads, and the
h recurrence scan chains naturally across the pair.

Engine plan per pair:
  PE    : 2 x 5 channel groups x 8 k-tiles (fp16, 1 cyc/row).  PSUM banks:
          sigmoid-fed groups rotate over 4 banks, v groups over the other 4
          (v is consumed latest), so the PE never waits on a bank.
  ACT   : 3 sigmoids per block ([a1|i1] share one bank+instr), pair-wide
          squares (in every act table) and sqrts.  A 1-element Copy reading
          the pair's last sigmoid output produces the sqrt bias tile (==1.0),
          forcing sqrts to schedule after the sigmoids: 2 table loads/pair.
  DVE   : u = i*v (PSUM reads); the chunked gate via ONE pair-wide scan:
          with M = a*mask (mask = 1e10 at chunk starts, 0 elsewhere),
          C[t] = max(a[t]*C[t-1], M[t]) equals 1e10 * within-chunk running
          product exactly (state<=1e10 so the max is a hard reset at chunk
          starts); then gw = min(C,1)*w in one fused scalar_tensor_tensor;
          finally the pair-wide h recurrence scan.
  Pool  : M = a*mask and w = r*u products (SBUF-only engine).
"""

import sys

if "/opt/trn_rl_repo" not in sys.path:
    sys.path.insert(0, "/opt/trn_rl_repo")

from contextlib import ExitStack

import numpy as np

from concourse import bacc, bass, mybir, tile
from concourse.bass_utils import run_bass_kernel_spmd

B, S = 4, 4096
DM, DR = 1024, 384
DC = DR // 2          # channels per core
CH = 64               # scan chunk size
SB = 512              # sequence block (one PSUM tile)
PB = 2 * SB           # pair block for SBUF-side work
NB = S // SB
KT = DM // 128        # contraction tiles
WC = 576              # stacked weight columns: a0|i0|v0|[a1,i1]|v1

F32 = mybir.dt.float32
F16 = mybir.dt.float16
AFT = mybir.ActivationFunctionType
OP = mybir.AluOpType

# column ranges of the stacked weight / PSUM group layout
GCOLS = ((0, 128), (128, 256), (256, 384), (384, 512), (512, 576))

_CACHED_NC = None


def _build_nc():
    nc = bacc.Bacc(trn_type="TRN2")

    xT = nc.dram_tensor("xt", [DM, S], F16, kind="ExternalInput")
    wT = nc.dram_tensor("wt", [DM, WC], F16, kind="ExternalInput")
    bias = nc.dram_tensor("biasa", [128, 2], F32, kind="ExternalInput")
    out = nc.dram_tensor("out", [DC, S], F16, kind="ExternalOutput")

    with tile.TileContext(nc) as tc, ExitStack() as ctx:
        wp = ctx.enter_context(tc.tile_pool(name="wp", bufs=1))
        cp = ctx.enter_context(tc.tile_pool(name="cp", bufs=1))
        xp = ctx.enter_context(tc.tile_pool(name="xp", bufs=4))
        pp = ctx.enter_context(tc.tile_pool(name="pp", bufs=1, space="PSUM"))
        sp = ctx.enter_context(tc.tile_pool(name="sp", bufs=2))
        hp = ctx.enter_context(tc.tile_pool(name="hp", bufs=2))

        # --- constants -------------------------------------------------
        # weights split into per-group DMAs so the first matmul can start
        # before the whole weight load finishes
        w_sb = wp.tile([128, KT, WC], F16, tag="w")
        for c0, c1 in GCOLS:
            nc.sync.dma_start(
                w_sb[:, :, c0:c1],
                wT.rearrange("(k p) c -> p k c", p=128)[:, :, c0:c1])

        # bias columns: col 0 = decay_bias[0:128]; col 1 = [bias[128:192]; 0]
        bt = cp.tile([128, 2], F32, tag="bt")
        nc.sync.dma_start(bt[:], bias[:, :])

        # chunk-reset mask: 1e10 at chunk-start columns, 0 elsewhere
        mask = cp.tile([128, PB], F32, tag="mask")
        nc.vector.memset(mask[:], 0.0)
        for c in range(PB // CH):
            nc.vector.memset(mask[:, c * CH:c * CH + 1], 1e10)

        def front_half(ib, half, a0p, ai1p, u0p, u1p):
            """DMA + matmuls + sigmoids + u for block ib (pair half 0/1)."""
            s0 = ib * SB
            cs = slice(half * SB, (half + 1) * SB)
            x_sb = xp.tile([128, KT, SB], F16, tag="x")
            nc.sync.dma_start(
                x_sb[:],
                xT.rearrange("(k p) s -> p k s", p=128)[:, :, s0:s0 + SB])

            # PSUM: sigmoid-fed groups (j 0,1,3) rotate over banks s0..s3,
            # v groups (j 2,4) over banks v0..v3 (freed latest by DVE u).
            zp = []
            for j, (c0, c1) in enumerate(GCOLS):
                if j in (0, 1, 3):
                    tag = f"s{(ib * 3 + (0, 1, None, 2, None)[j]) % 4}"
                else:
                    tag = f"v{(ib * 2 + (None, None, 0, None, 1)[j]) % 4}"
                zt = pp.tile([128, SB], F32, tag=tag)
                z = zt[0:c1 - c0, :]
                for k in range(KT):
                    nc.tensor.matmul(
                        z,
                        w_sb[:, k, c0:c1],
                        x_sb[:, k, :],
                        start=(k == 0),
                        stop=(k == KT - 1),
                    )
                zp.append(z)
            za0, zi0, zv0, zai1, zv1 = zp

            i0 = sp.tile([128, PB], F16, tag="i0")
            nc.scalar.activation(a0p[:, cs], za0, AFT.Sigmoid, bias=bt[:, 0:1])
            nc.scalar.activation(i0[:, cs], zi0, AFT.Sigmoid)
            # one sigmoid for the [a1|i1] bank; bias col1 = [b1;0]
            nc.scalar.activation(ai1p[:, cs], zai1, AFT.Sigmoid, bias=bt[:, 1:2])

            # u = i*v on DVE (reads v straight from PSUM, frees the bank)
            nc.vector.tensor_tensor(u0p[:, cs], i0[:, cs], zv0, OP.mult)
            nc.vector.tensor_tensor(
                u1p[:, cs], ai1p[64:128, cs], zv1[0:64, :], OP.mult)
            return ai1p

        prev_h = None
        for p in range(NB // 2):
            # pair-wide fp16 tiles: [:, 0:512] = block A, [:, 512:1024] = B
            a0p = sp.tile([128, PB], F16, tag="a0p")
            ai1p = sp.tile([128, PB], F16, tag="ai1p")
            u0p = sp.tile([128, PB], F16, tag="u0p")
            u1p = sp.tile([64, PB], F16, tag="u1p")

            front_half(2 * p, 0, a0p, ai1p, u0p, u1p)
            front_half(2 * p + 1, 1, a0p, ai1p, u0p, u1p)

            # 1-element Copy reading the pair's last sigmoid output; produces
            # the all-ones sqrt bias column and pins sqrts after sigmoids.
            gate = sp.tile([128, 1], F32, tag="gate")
            nc.scalar.activation(
                gate[:], ai1p[:, PB - 1:PB], AFT.Copy, bias=1.0, scale=0.0)

            a1p = ai1p[0:64, :]
            new_h = {}
            for gi, (ap, up, P) in enumerate(
                    ((a0p, u0p, 128), (a1p, u1p, 64))):
                # m = a*a on ACT: Square is in every act table set
                m = sp.tile([P, PB], F16, tag=f"m{gi}")
                nc.scalar.activation(m[:], ap, AFT.Square)
                r = sp.tile([P, PB], F16, tag=f"r{gi}")
                # r = sqrt(gate*1 - m) = sqrt(1 - a*a)
                nc.scalar.activation(
                    r[:], m[:], AFT.Sqrt, bias=gate[0:P, :], scale=-1.0)
                w = sp.tile([P, PB], F16, tag=f"w{gi}")
                nc.gpsimd.tensor_tensor(w[:], r[:], up, OP.mult)

                # C = 1e10 * within-chunk running product of a, via one scan
                mm = sp.tile([P, PB], F32, tag=f"mm{gi}")
                nc.gpsimd.tensor_tensor(mm[:], ap, mask[0:P, :], OP.mult)
                cc = sp.tile([P, PB], F32, tag=f"cc{gi}")
                nc.vector.tensor_tensor_scan(
                    cc[:], ap, mm[:], 0.0, op0=OP.mult, op1=OP.max
                )
                # gw = min(C, 1) * w  (fused)
                gw = sp.tile([P, PB], F16, tag=f"gw{gi}")
                nc.vector.scalar_tensor_tensor(
                    gw[:], cc[:], 1.0, w[:], op0=OP.min, op1=OP.mult
                )
                h = hp.tile([P, PB], F16, tag=f"h{gi}")
                init = 0.0 if prev_h is None else prev_h[gi][:, PB - 1:PB]
                nc.vector.tensor_tensor_scan(
                    h[:], ap, gw[:], init, op0=OP.mult, op1=OP.add
                )
                c0 = 0 if gi == 0 else 128
                nc.sync.dma_start(
                    out[c0:c0 + P, 2 * p * SB:(2 * p + 2) * SB], h[:])
                new_h[gi] = h
            prev_h = new_h

    nc.finalize()
    return nc


def _make_in_maps(x, Wa, Wi, Wv, decay_bias):
    x = np.asarray(x, dtype=np.float32)
    Wa = np.asarray(Wa, dtype=np.float32)
    Wi = np.asarray(Wi, dtype=np.float32)
    Wv = np.asarray(Wv, dtype=np.float32)
    decay_bias = np.asarray(decay_bias, dtype=np.float32)

    in_maps = []
    for b in range(B):
        xTb = np.ascontiguousarray(x[b].T.astype(np.float16))   # [DM, S]
        for j in range(2):
            c0 = j * DC
            # stacked weight [DM, 576]: a0 | i0 | v0 | a1,i1 | v1
            wcat = np.concatenate([
                Wa[c0:c0 + 128].T,
                Wi[c0:c0 + 128].T,
                Wv[c0:c0 + 128].T,
                Wa[c0 + 128:c0 + DC].T,
                Wi[c0 + 128:c0 + DC].T,
                Wv[c0 + 128:c0 + DC].T,
            ], axis=1).astype(np.float16)
            bcols = np.zeros((128, 2), dtype=np.float32)
            bcols[:, 0] = decay_bias[c0:c0 + 128]
            bcols[0:64, 1] = decay_bias[c0 + 128:c0 + DC]
            in_maps.append({
                "xt": xTb,
                "wt": np.ascontiguousarray(wcat),
                "biasa": bcols,
            })
    return in_maps


def kernel(x, Wa, Wi, Wv, decay_bias):
    global _CACHED_NC
    if _CACHED_NC is None:
        _CACHED_NC = _build_nc()
    nc = _CACHED_NC

    in_maps = _make_in_maps(x, Wa, Wi, Wv, decay_bias)
    res = run_bass_kernel_spmd(nc, in_maps, core_ids=list(range(8)))

    out = np.empty((B, S, DR), dtype=np.float32)
    for b in range(B):
        for j in range(2):
            core = 2 * b + j
            out[b, :, j * DC:(j + 1) * DC] = \
                res.results[core]["out"].T.astype(np.float32)
    return out


# revision 12
# speedup vs baseline: 1.3697x; 1.0995x over previous
"""Trainium2 Bass kernel for the Griffin-style gated linear recurrence.

Model (matching the jax reference, including its chunked-scan numerics):
    a = sigmoid(x @ Wa.T + decay_bias)
    i = sigmoid(x @ Wi.T)
    v = x @ Wv.T
    w = sqrt(max(1 - a*a, 1e-8)) * i * v
    chunked scan (chunk=64): equivalent to h[t] = a[t]*h[t-1] + g[t]*w[t]
    with g[t] = min(1, cd[t]*1e10), cd = within-chunk running product of a.

Sharding: 4 batches x 2 channel-halves = 8 cores, no communication.
Per core: x[b] as [1024, 4096] fp16, stacked weight shard [1024, 576] fp16
(cols: a0|i0|v0|[a1,i1]|v1), output [192, 4096] fp16 (host upcasts).

Blocks of 512 steps are processed in PAIRS; all SBUF-side elementwise work
runs on pair-wide [P, 1024] tiles to halve instruction overheads, and the
h recurrence scan chains naturally across the pair.

Engine plan per pair:
  PE    : 2 x 5 channel groups x 8 k-tiles (fp16, 1 cyc/row).  PSUM banks:
          sigmoid-fed groups rotate over 4 banks, v groups over the other 4
          (v is consumed latest), so the PE never waits on a bank.
  ACT   : 3 sigmoids per block ([a1|i1] share one bank+instr), pair-wide
          squares (in every act table) and sqrts.  A 1-element Copy reading
          the pair's last sigmoid output produces the sqrt bias tile (==1.0),
          forcing sqrts to schedule after the sigmoids: 2 table loads/pair.
  DVE   : u = i*v (PSUM reads); the chunked gate via ONE pair-wide scan:
          with M = a*mask (mask = 1e10 at chunk starts, 0 elsewhere),
          C[t] = max(a[t]*C[t-1], M[t]) equals 1e10 * within-chunk running
          product exactly (state<=1e10 so the max is a hard reset at chunk
          starts); then gw = min(C,1)*w in one fused scalar_tensor_tensor;
          finally the pair-wide h recurrence scan.
  Pool  : M = a*mask and w = r*u products (SBUF-only engine).
"""

import sys

if "/opt/trn_rl_repo" not in sys.path:
    sys.path.insert(0, "/opt/trn_rl_repo")

from contextlib import ExitStack

import numpy as np

from concourse import bacc, bass, mybir, tile
from concourse.bass_utils import run_bass_kernel_spmd

B, S = 4, 4096
DM, DR = 1024, 384
DC = DR // 2          # channels per core
CH = 64               # scan chunk size
SB = 512              # sequence block (one PSUM tile)
PB = 2 * SB           # pair block for SBUF-side work
NB = S // SB
KT = DM // 128        # contraction tiles
WC = 576              # stacked weight columns: a0|i0|v0|[a1,i1]|v1

F32 = mybir.dt.float32
F16 = mybir.dt.float16
AFT = mybir.ActivationFunctionType
OP = mybir.AluOpType

# column ranges of the stacked weight / PSUM group layout
GCOLS = ((0, 128), (128, 256), (256, 384), (384, 512), (512, 576))

_CACHED_NC = None


def _build_nc():
    nc = bacc.Bacc(trn_type="TRN2")

    xT = nc.dram_tensor("xt", [DM, S], F16, kind="ExternalInput")
    wT = nc.dram_tensor("wt", [DM, WC], F16, kind="ExternalInput")
    bias = nc.dram_tensor("biasa", [128, 2], F32, kind="ExternalInput")
    out = nc.dram_tensor("out", [DC, S], F16, kind="ExternalOutput")

    with tile.TileContext(nc) as tc, ExitStack() as ctx:
        wp = ctx.enter_context(tc.tile_pool(name="wp", bufs=1))
        cp = ctx.enter_context(tc.tile_pool(name="cp", bufs=1))
        xp = ctx.enter_context(tc.tile_pool(name="xp", bufs=4))
        pp = ctx.enter_context(tc.tile_pool(name="pp", bufs=1, space="PSUM"))
        sp = ctx.enter_context(tc.tile_pool(name="sp", bufs=2))
        hp = ctx.enter_context(tc.tile_pool(name="hp", bufs=2))

        # --- x prefetch + constants ------------------------------------
        # first x block goes out before the weights so the PE can start
        # as soon as both arrive; remaining blocks stream behind.
        x_tiles = {}

        def fetch_x(ib):
            x_sb = xp.tile([128, KT, SB], F16, tag="x", name=f"x{ib}")
            nc.sync.dma_start(
                x_sb[:],
                xT.rearrange("(k p) s -> p k s", p=128)
                [:, :, ib * SB:(ib + 1) * SB])
            x_tiles[ib] = x_sb

        fetch_x(0)
        w_sb = wp.tile([128, KT, WC], F16, tag="w")
        nc.sync.dma_start(w_sb[:], wT.rearrange("(k p) c -> p k c", p=128))
        for ib in range(1, NB):
            fetch_x(ib)

        # bias columns: col 0 = decay_bias[0:128]; col 1 = [bias[128:192]; 0]
        bt = cp.tile([128, 2], F32, tag="bt")
        nc.sync.dma_start(bt[:], bias[:, :])



        def front_half(ib, half, a0p, ai1p, u0p, u1p):
            """Matmuls + sigmoids + u for block ib (pair half 0/1)."""
            cs = slice(half * SB, (half + 1) * SB)
            x_sb = x_tiles[ib]

            # PSUM: sigmoid-fed groups (j 0,1,3) rotate over banks s0..s3,
            # v groups (j 2,4) over banks v0..v3 (freed latest by DVE u).
            zp = []
            for j, (c0, c1) in enumerate(GCOLS):
                if j in (0, 1, 3):
                    tag = f"s{(ib * 3 + (0, 1, None, 2, None)[j]) % 4}"
                else:
                    tag = f"v{(ib * 2 + (None, None, 0, None, 1)[j]) % 4}"
                zt = pp.tile([128, SB], F32, tag=tag)
                z = zt[0:c1 - c0, :]
                for k in range(KT):
                    nc.tensor.matmul(
                        z,
                        w_sb[:, k, c0:c1],
                        x_sb[:, k, :],
                        start=(k == 0),
                        stop=(k == KT - 1),
                    )
                zp.append(z)
            za0, zi0, zv0, zai1, zv1 = zp

            i0 = sp.tile([128, PB], F16, tag="i0")
            nc.scalar.activation(a0p[:, cs], za0, AFT.Sigmoid, bias=bt[:, 0:1])
            nc.scalar.activation(i0[:, cs], zi0, AFT.Sigmoid)
            # one sigmoid for the [a1|i1] bank; bias col1 = [b1;0]
            nc.scalar.activation(ai1p[:, cs], zai1, AFT.Sigmoid, bias=bt[:, 1:2])

            # u = i*v on DVE (reads v straight from PSUM, frees the bank)
            nc.vector.tensor_tensor(u0p[:, cs], i0[:, cs], zv0, OP.mult)
            nc.vector.tensor_tensor(
                u1p[:, cs], ai1p[64:128, cs], zv1[0:64, :], OP.mult)
            return ai1p

        prev_h = None
        for p in range(NB // 2):
            # pair-wide fp16 tiles: [:, 0:512] = block A, [:, 512:1024] = B
            a0p = sp.tile([128, PB], F16, tag="a0p")
            ai1p = sp.tile([128, PB], F16, tag="ai1p")
            u0p = sp.tile([128, PB], F16, tag="u0p")
            u1p = sp.tile([64, PB], F16, tag="u1p")

            front_half(2 * p, 0, a0p, ai1p, u0p, u1p)
            front_half(2 * p + 1, 1, a0p, ai1p, u0p, u1p)

            # 1-element Copy reading the pair's last sigmoid output; produces
            # the all-ones sqrt bias column and pins sqrts after sigmoids.
            gate = sp.tile([128, 1], F32, tag="gate")
            nc.scalar.activation(
                gate[:], ai1p[:, PB - 1:PB], AFT.Copy, bias=1.0, scale=0.0)

            a1p = ai1p[0:64, :]
            # the last pair runs its back-end in two 512-wide halves to
            # shorten the end-of-kernel drain chain
            col_slices = ([slice(0, PB)] if p < NB // 2 - 1
                          else [slice(0, SB), slice(SB, PB)])
            new_h = {}
            for gi, (ap, up, P) in enumerate(
                    ((a0p, u0p, 128), (a1p, u1p, 64))):
                # m = a*a on ACT: Square is in every act table set
                m = sp.tile([P, PB], F16, tag=f"m{gi}")
                r = sp.tile([P, PB], F16, tag=f"r{gi}")
                w = sp.tile([P, PB], F16, tag=f"w{gi}")
                mm = sp.tile([P, PB], F32, tag=f"mm{gi}")
                cc = sp.tile([P, PB], F32, tag=f"cc{gi}")
                gw = sp.tile([P, PB], F16, tag=f"gw{gi}")
                h = hp.tile([P, PB], F16, tag=f"h{gi}")
                # M (= a*1e10 at chunk starts, 0 elsewhere) only needs its
                # reset columns written: the mm buffers are zeroed on first
                # use (first two pairs of each tag), then only the strided
                # columns are updated.
                if p < 2:
                    nc.vector.memset(mm[:], 0.0)
                prev = None if prev_h is None else prev_h[gi]
                for cs in col_slices:
                    nc.scalar.activation(m[:, cs], ap[:, cs], AFT.Square)
                    # r = sqrt(gate*1 - m) = sqrt(1 - a*a)
                    nc.scalar.activation(
                        r[:, cs], m[:, cs], AFT.Sqrt,
                        bias=gate[0:P, :], scale=-1.0)
                    nc.gpsimd.tensor_tensor(
                        w[:, cs], r[:, cs], up[:, cs], OP.mult)
                    # C = 1e10 * within-chunk running product of a (one scan)
                    av = ap[:, cs].rearrange("q (c u) -> q c u", u=CH)[:, :, 0]
                    mv = mm[:, cs].rearrange("q (c u) -> q c u", u=CH)[:, :, 0]
                    nc.vector.tensor_scalar(mv, av, 1e10, None, op0=OP.mult)
                    nc.vector.tensor_tensor_scan(
                        cc[:, cs], ap[:, cs], mm[:, cs], 0.0,
                        op0=OP.mult, op1=OP.max
                    )
                    # gw = min(C, 1) * w  (fused)
                    nc.vector.scalar_tensor_tensor(
                        gw[:, cs], cc[:, cs], 1.0, w[:, cs],
                        op0=OP.min, op1=OP.mult
                    )
                    init = 0.0 if prev is None else prev[:, cs.start - 1:cs.start] \
                        if cs.start > 0 else prev[:, PB - 1:PB]
                    nc.vector.tensor_tensor_scan(
                        h[:, cs], ap[:, cs], gw[:, cs], init,
                        op0=OP.mult, op1=OP.add
                    )
                    prev = h
                    c0 = 0 if gi == 0 else 128
                    nc.sync.dma_start(
                        out[c0:c0 + P,
                            2 * p * SB + cs.start:2 * p * SB + cs.stop],
                        h[:, cs])
                new_h[gi] = h
            prev_h = new_h

    nc.finalize()
    return nc


def _make_in_maps(x, Wa, Wi, Wv, decay_bias):
    x = np.asarray(x, dtype=np.float32)
    Wa = np.asarray(Wa, dtype=np.float32)
    Wi = np.asarray(Wi, dtype=np.float32)
    Wv = np.asarray(Wv, dtype=np.float32)
    decay_bias = np.asarray(decay_bias, dtype=np.float32)

    in_maps = []
    for b in range(B):
        xTb = np.ascontiguousarray(x[b].T.astype(np.float16))   # [DM, S]
        for j in range(2):
            c0 = j * DC
            # stacked weight [DM, 576]: a0 | i0 | v0 | a1,i1 | v1
            wcat = np.concatenate([
                Wa[c0:c0 + 128].T,
                Wi[c0:c0 + 128].T,
                Wv[c0:c0 + 128].T,
                Wa[c0 + 128:c0 + DC].T,
                Wi[c0 + 128:c0 + DC].T,
                Wv[c0 + 128:c0 + DC].T,
            ], axis=1).astype(np.float16)
            bcols = np.zeros((128, 2), dtype=np.float32)
            bcols[:, 0] = decay_bias[c0:c0 + 128]
            bcols[0:64, 1] = decay_bias[c0 + 128:c0 + DC]
            in_maps.append({
                "xt": xTb,
                "wt": np.ascontiguousarray(wcat),
                "biasa": bcols,
            })
    return in_maps


def kernel(x, Wa, Wi, Wv, decay_bias):
    global _CACHED_NC
    if _CACHED_NC is None:
        _CACHED_NC = _build_nc()
    nc = _CACHED_NC

    in_maps = _make_in_maps(x, Wa, Wi, Wv, decay_bias)
    res = run_bass_kernel_spmd(nc, in_maps, core_ids=list(range(8)))

    out = np.empty((B, S, DR), dtype=np.float32)
    for b in range(B):
        for j in range(2):
            core = 2 * b + j
            out[b, :, j * DC:(j + 1) * DC] = \
                res.results[core]["out"].T.astype(np.float32)
    return out
